# revision 11
# baseline (speedup 1.0000x reference)
"""Trainium2 Bass kernel for nn_GCNII_80178449482260 (2x dense GAT + GCNII).

Distribution: row-parallel over the node dimension N=1024 across 8 cores
(128 rows each). Every attention layer (5 GAT1 heads, GAT1 out, 2 GAT2
heads, GAT2 out) runs the same schema:

    Wh_rows = x_rows @ W           (PE, bf16, fp32 accum)
    u, v    = Wh_rows @ a1/a2      (DVE fused mult+accum)
    AllGather[Wh_rows | v]         (one bf16 collective per layer)
    n       = exp(lrelu_{0.01}(u + v^T + mask))   (ACT, fused row-sum)
    out     = elu((1/sum) * (n @ Wh_full))        (PE + DVE/ACT)
    outT    via PE transposes -> next layer's lhsT

The GCNII tail (dead layer 1 elided) reuses the gathered h0/support blocks.
All matmuls bf16 with fp32 PSUM; elu keeps exp in fp32 (expm1 cancellation).

Self-contained: builds/compiles the Bass program on first call, caches it,
and runs via run_bass_kernel_spmd on cores 0-7.
"""
import os
import sys
import numpy as np

for _p in ("/opt/trn_rl_repo", "/root/.axon_site/_ro/trn_rl_repo"):
    if _p not in sys.path:
        sys.path.insert(0, _p)

import ml_dtypes  # noqa: E402
from concourse import bacc, tile, mybir  # noqa: E402
from concourse.bass_utils import run_bass_kernel_spmd  # noqa: E402
from concourse.kernels.tile_matmul import make_identity  # noqa: E402

BF16 = mybir.dt.bfloat16
F32 = mybir.dt.float32
AF = mybir.ActivationFunctionType
OP = mybir.AluOpType

N = 1024      # nodes
P = 128       # partitions / rows per core
C = 8         # cores
HID = 512
NC1 = 512
H1, H2 = 5, 2
PAD = 16      # v + alignment pad columns appended to the Wh allgather
THETA2 = float(np.log(1.25))   # GCNII layer-2 theta; layer 1 is dead code
SLOPE = 0.25
RG = [list(range(C))]
_NO_CC = bool(int(os.environ.get("KERNEL_NO_CC", "0")))  # profiling stand-in

_CACHE = {}


def _build(reps=1):
    nc = bacc.Bacc("TRN2", target_bir_lowering=False, debug=False,
                   num_devices=C)
    d = {}
    d["xT_sl"] = nc.dram_tensor("xT_sl", [N, P], BF16, kind="ExternalInput")
    d["adj_r"] = nc.dram_tensor("adj_r", [P, N], F32, kind="ExternalInput")
    d["Wg1"] = nc.dram_tensor("Wg1", [H1, N, N], BF16, kind="ExternalInput")
    d["ag1"] = nc.dram_tensor("ag1", [H1, 2 * N], BF16, kind="ExternalInput")
    d["Wo1"] = nc.dram_tensor("Wo1", [H1 * N, NC1], BF16, kind="ExternalInput")
    d["ao1"] = nc.dram_tensor("ao1", [2 * NC1], BF16, kind="ExternalInput")
    d["Wg2"] = nc.dram_tensor("Wg2", [H2, NC1, NC1], BF16, kind="ExternalInput")
    d["ag2"] = nc.dram_tensor("ag2", [H2, 2 * NC1], BF16, kind="ExternalInput")
    d["Wo2"] = nc.dram_tensor("Wo2", [N, N], BF16, kind="ExternalInput")
    d["ao2"] = nc.dram_tensor("ao2", [2 * N], BF16, kind="ExternalInput")
    d["fc0_w"] = nc.dram_tensor("fc0_w", [N, HID], BF16, kind="ExternalInput")
    d["fc0_b"] = nc.dram_tensor("fc0_b", [HID], BF16, kind="ExternalInput")
    d["fc1_w"] = nc.dram_tensor("fc1_w", [HID, N], BF16, kind="ExternalInput")
    d["fc1_b"] = nc.dram_tensor("fc1_b", [N], BF16, kind="ExternalInput")
    d["cw1T_sl"] = nc.dram_tensor("cw1T_sl", [N, P], BF16, kind="ExternalInput")
    out_d = nc.dram_tensor("out", [P, N], F32, kind="ExternalOutput")

    with tile.TileContext(nc) as tc:
        _body(nc, tc, d, out_d, reps)
    nc.compile()
    return nc


def _body(nc, tc, d, out_d, reps=1):
    def _chunked_dma(dst3d, src_ap_3d, nchunks, nsplit=4):
        """DMA [p, nchunks, L] in nsplit pieces so multiple queues engage."""
        step = max(1, nchunks // nsplit)
        c = 0
        while c < nchunks:
            e = min(nchunks, c + step)
            nc.sync.dma_start(dst3d[:, c:e, :], src_ap_3d[:, c:e, :])
            c = e

    with (
        tc.tile_pool(name="cst", bufs=1) as cst,
        tc.tile_pool(name="per", bufs=1) as per,        # cross-phase persistents
        tc.tile_pool(name="whfp", bufs=2) as whf_p,     # gathered Wh_full tiles
        tc.tile_pool(name="wstr", bufs=2) as w_str,     # small resident weights
        tc.tile_pool(name="wch", bufs=8) as wch_p,      # weight chunk stream
        tc.tile_pool(name="attp", bufs=2) as att_p,     # attention work
        tc.tile_pool(name="scbf", bufs=2) as sc_bf,     # bf16 scratch
        tc.tile_pool(name="sc32", bufs=2) as sc_32,     # f32 scratch
        tc.tile_pool(name="smv", bufs=2) as sm_vec,     # [1, *] vectors
        tc.tile_pool(name="smt", bufs=4) as sm,         # tiny per-row vecs
        tc.tile_pool(name="abcp", bufs=2) as abc_p,     # a-vec bcasts
        tc.tile_pool(name="pswh", bufs=2, space="PSUM") as ps_wh,
        tc.tile_pool(name="psout", bufs=1, space="PSUM") as ps_out,
        tc.tile_pool(name="pstr", bufs=2, space="PSUM") as ps_tr,
        tc.tile_pool(name="dram", bufs=1, space="DRAM") as dram,
    ):
        ident = cst.tile([P, P], BF16, tag="ident")
        make_identity(nc, ident)

        xT_sb = per.tile([P, C, P], BF16, tag="xtsl")    # x_rows^T, k-chunk c
        nc.sync.dma_start(xT_sb[:], d["xT_sl"].ap().rearrange("(c p) m -> p c m", p=P))

        adj_sb = sc_32.tile([P, N], F32, tag="s32")
        nc.sync.dma_start(adj_sb[:], d["adj_r"].ap())
        madj = per.tile([P, N], BF16, tag="madj")        # 0 where adj>0 else -9e15
        nc.vector.tensor_scalar(madj[:], adj_sb[:], 0.0, None, op0=OP.is_gt)
        nc.vector.tensor_scalar(madj[:], madj[:], 1.0, 9e15,
                                op0=OP.subtract, op1=OP.mult)

        # ---------------- helpers ----------------
        def bcast_vec(dst_bf, src_dram_ap, L):
            a_sb = sm_vec.tile([1, 2 * N], BF16, tag="avec")
            nc.gpsimd.dma_start(a_sb[:1, :L], src_dram_ap[None, :])
            nc.gpsimd.partition_broadcast(dst_bf[:, :L], a_sb[:1, :L])

        def w_stream(src_3d_ap, L):
            """Returns rhs_fn(c) that DMAs chunk c of a DRAM [*, nk, L] view."""
            def fn(c):
                t = wch_p.tile([P, N], BF16, tag="wch")
                nc.sync.dma_start(t[:, :L], src_3d_ap[:, c, :])
                return t[:, :L]
            return fn

        def allgather(src_sb, rows, cols, dtype, tag):
            ag_in = dram.tile([rows, cols], dtype, tag=f"agi_{tag}")
            ag_out = dram.tile([C * rows, cols], dtype, tag=f"ago_{tag}")
            nc.gpsimd.dma_start(ag_in[:], src_sb)
            if _NO_CC:
                for cc in range(C):
                    nc.gpsimd.dma_start(ag_out[cc * rows:(cc + 1) * rows, :], ag_in[:])
            else:
                nc.gpsimd.collective_compute(
                    "AllGather", OP.bypass, replica_groups=RG,
                    ins=[ag_in.opt()], outs=[ag_out.opt()])
            return ag_out

        def copy_ps(dst, src, idx=0):
            if idx % 2 == 0:
                nc.scalar.activation(dst, src, AF.Copy)
            else:
                nc.vector.tensor_copy(dst, src)

        def elu_inplace(x_bf, L):
            """x <- elu(x) on [128, L] bf16 (fp32 exp path for expm1)."""
            m_bf = sc_bf.tile([P, N], BF16, tag="elum")
            nc.vector.tensor_scalar(m_bf[:, :L], x_bf, 0.0, None, op0=OP.min)
            g32 = sc_32.tile([P, N], F32, tag="s32")
            nc.scalar.activation(g32[:, :L], m_bf[:, :L], AF.Exp)
            r32 = sc_32.tile([P, N], F32, tag="s32b")
            nc.vector.tensor_scalar(r32[:, :L], x_bf, 0.0, None, op0=OP.max)
            nc.vector.scalar_tensor_tensor(x_bf, g32[:, :L], -1.0, r32[:, :L],
                                           op0=OP.add, op1=OP.add)

        def transpose_into(dst_3d, src_bf, nchunks, off=0):
            for j in range(nchunks):
                tp = ps_tr.tile([P, P], BF16, tag="tr")
                nc.tensor.transpose(tp[:], src_bf[:, j * P:(j + 1) * P], ident[:])
                copy_ps(dst_3d[:, off + j, :], tp[:], j)

        def gat_layer(lhsT_sb, lhs_idx, rhs_fn, L, a_dram, tagid,
                      out_T, out_T_off=0):
            """One attention layer (see module docstring).

            lhsT_sb[:, c, :] [128, 128] for c in lhs_idx; rhs_fn(c) -> [128, L]
            SBUF AP; a_dram: [2L] bf16 DRAM vector. Writes the elu'd transposed
            output into out_T (3d [P, *, P]) starting at chunk out_T_off."""
            nk = len(lhs_idx)
            # Wh = x_rows @ W
            wh = ps_wh.tile([P, N], F32, tag="whps")
            for i, c in enumerate(lhs_idx):
                rhs = rhs_fn(c)
                for s in range(0, L, 512):
                    w = min(512, L - s)
                    nc.tensor.matmul(wh[:, s:s + w], lhsT_sb[:, c, :],
                                     rhs[:, s:s + w],
                                     start=(i == 0), stop=(i == nk - 1))
            # u, v
            a_bc = abc_p.tile([P, 2 * N], BF16, tag="abc")
            bcast_vec(a_bc, a_dram, 2 * L)
            uv = sm.tile([P, 2], F32, tag=f"uv_{tagid}")
            junk = sc_bf.tile([P, N], BF16, tag="junk")
            nc.vector.scalar_tensor_tensor(junk[:, :L], wh[:, :L], 1.0,
                                           a_bc[:, :L], op0=OP.mult, op1=OP.mult,
                                           accum_out=uv[:, 0:1])
            nc.vector.scalar_tensor_tensor(junk[:, :L], wh[:, :L], 1.0,
                                           a_bc[:, L:2 * L], op0=OP.mult,
                                           op1=OP.mult, accum_out=uv[:, 1:2])
            # [Wh | v] -> bf16, allgather
            whv = sc_bf.tile([P, N + PAD], BF16, tag="whv")
            nc.scalar.activation(whv[:, :L], wh[:, :L], AF.Copy)
            nc.vector.tensor_scalar(whv[:, L:L + PAD],
                                    uv[:, 1:2].to_broadcast((P, PAD)),
                                    1.0, None, op0=OP.mult)
            ag = allgather(whv[:, :L + PAD], P, L + PAD, BF16, f"whv_{tagid}")
            # attention: n = exp(lrelu(u + v^T + madj)), s = rowsum
            v_sb = sm_vec.tile([1, N], BF16, tag="vfull")
            nc.gpsimd.dma_start(
                v_sb[:], ag[:, L:L + 1].rearrange("(c p) o -> o (c p)", p=P))
            vb = att_p.tile([P, N], BF16, tag="vb")
            nc.gpsimd.partition_broadcast(vb[:], v_sb[:1, :])
            e_sb = sc_32.tile([P, N], F32, tag="s32")
            nc.vector.scalar_tensor_tensor(e_sb[:], vb[:], uv[:, 0:1], madj[:],
                                           op0=OP.add, op1=OP.add)
            el = sc_32.tile([P, N], F32, tag="s32b")
            nc.scalar.activation(el[:], e_sb[:], AF.Lrelu)   # slope fixed 0.01
            n_bf = att_p.tile([P, N], BF16, tag="nbf")
            ssum = sm.tile([P, 1], F32, tag="ssum")
            nc.scalar.activation(n_bf[:], el[:], AF.Exp, accum_out=ssum[:])
            rs = sm.tile([P, 1], F32, tag=f"rs_{tagid}")
            nc.vector.reciprocal(rs[:], ssum[:])
            # attT
            attT = att_p.tile([P, C, P], BF16, tag="attT")
            for j in range(C):
                tp = ps_tr.tile([P, P], BF16, tag="tr")
                nc.tensor.transpose(tp[:], n_bf[:, j * P:(j + 1) * P], ident[:])
                copy_ps(attT[:, j, :], tp[:], j)
            # Wh_full pull (strided past the pad columns)
            wh_full = whf_p.tile([P, C, N], BF16, tag="whfull")
            ag_v3 = ag[:, :L].rearrange("(c p) f -> p c f", p=P)
            for j0 in range(0, C, 2):
                nc.sync.dma_start(wh_full[:, j0:j0 + 2, :L], ag_v3[:, j0:j0 + 2, :])
            # out = rs * (n @ Wh_full), elu, transpose
            o_ps = ps_out.tile([P, N], F32, tag="ops")
            for j in range(C):
                for s in range(0, L, 512):
                    w = min(512, L - s)
                    nc.tensor.matmul(o_ps[:, s:s + w], attT[:, j, :],
                                     wh_full[:, j, s:s + w],
                                     start=(j == 0), stop=(j == C - 1))
            o_bf = sc_bf.tile([P, N], BF16, tag="obf")
            nc.vector.tensor_scalar(o_bf[:, :L], o_ps[:, :L], rs[:], None,
                                    op0=OP.mult)
            elu_inplace(o_bf[:, :L], L)
            transpose_into(out_T, o_bf[:], L // P, off=out_T_off)

        hcatT = per.tile([P, H1 * C, P], BF16, tag="hcatT")   # [128, 40, 128]
        xgT = per.tile([P, 4, P], BF16, tag="xgT")
        hcat2T = per.tile([P, C, P], BF16, tag="hcat2T")
        xg2T = per.tile([P, C, P], BF16, tag="xg2T")

        h0f = per.tile([P, HID], F32, tag="h0f")
        h0_full = per.tile([P, C, HID], BF16, tag="h0full")

        for _rep in range(reps):
            # ======== GCNII h0 (independent -> overlaps GAT compute) ========
            fc0_fn = w_stream(d["fc0_w"].ap().rearrange("(c p) f -> p c f", p=P), HID)
            h0_ps = ps_wh.tile([P, N], F32, tag="whps")
            for c in range(C):
                nc.tensor.matmul(h0_ps[:, :HID], xT_sb[:, c, :], fc0_fn(c),
                                 start=(c == 0), stop=(c == C - 1))
            b_bc = abc_p.tile([P, 2 * N], BF16, tag="abc")
            bcast_vec(b_bc, d["fc0_b"].ap(), HID)
            nc.vector.scalar_tensor_tensor(h0f[:], h0_ps[:, :HID], 1.0, b_bc[:, :HID],
                                           op0=OP.mult, op1=OP.add)
            nc.vector.scalar_tensor_tensor(h0f[:], h0f[:], SLOPE, h0f[:],
                                           op0=OP.mult, op1=OP.max)
            h0b = sc_bf.tile([P, HID], BF16, tag="h0b")
            nc.scalar.activation(h0b[:], h0f[:], AF.Copy)
            ag_h0 = allgather(h0b[:], P, HID, BF16, "h0")

            # ================= GAT1: 5 heads =================
            for h in range(H1):
                gat_layer(xT_sb, range(C),
                          w_stream(d["Wg1"].ap()[h].rearrange("(c p) f -> p c f", p=P), N),
                          N, d["ag1"].ap()[h], f"g1_{h}",
                          out_T=hcatT, out_T_off=h * C)

            # ================= GAT1 out-attention =================
            gat_layer(hcatT, range(H1 * C),
                      w_stream(d["Wo1"].ap().rearrange("(c p) f -> p c f", p=P), NC1),
                      NC1, d["ao1"].ap(), "o1", out_T=xgT)

            # ================= GAT2: 2 heads =================
            wg2_sb = w_str.tile([P, H2, 4, NC1], BF16, tag="wg2")
            for h in range(H2):
                nc.sync.dma_start(wg2_sb[:, h], d["Wg2"].ap()[h].rearrange("(c p) f -> p c f", p=P))
            for h in range(H2):
                gat_layer(xgT, range(4), lambda c, t=wg2_sb, hh=h: t[:, hh, c, :], NC1,
                          d["ag2"].ap()[h], f"g2_{h}",
                          out_T=hcat2T, out_T_off=h * 4)

            # ================= GAT2 out-attention =================
            gat_layer(hcat2T, range(C),
                      w_stream(d["Wo2"].ap().rearrange("(c p) f -> p c f", p=P), N),
                      N, d["ao2"].ap(), "o2", out_T=xg2T)

            # ================= GCNII =================
            nc.sync.dma_start(h0_full[:], ag_h0[:].rearrange("(c p) f -> p c f", p=P))
            hi_ps = ps_wh.tile([P, N], F32, tag="whps")
            for j in range(C):
                nc.tensor.matmul(hi_ps[:, :HID], xg2T[:, j, :], h0_full[:, j, :],
                                 start=(j == 0), stop=(j == C - 1))
            sf = per.tile([P, HID], F32, tag="sf")
            nc.vector.scalar_tensor_tensor(sf[:], hi_ps[:, :HID], 9.0, h0f[:],
                                           op0=OP.mult, op1=OP.add)
            nc.vector.tensor_scalar(sf[:], sf[:], 0.1, None, op0=OP.mult)
            sb_bf = sc_bf.tile([P, HID], BF16, tag="h0b")
            nc.scalar.activation(sb_bf[:], sf[:], AF.Copy)
            ag_s = allgather(sb_bf[:], P, HID, BF16, "s")
            s_full = whf_p.tile([P, C, N], BF16, tag="whfull")
            nc.sync.dma_start(s_full[:, :, :HID],
                              ag_s[:].rearrange("(c p) f -> p c f", p=P))

            cw1T_sb = w_str.tile([P, C, P], BF16, tag="wg2")
            nc.sync.dma_start(cw1T_sb[:], d["cw1T_sl"].ap().rearrange("(c p) m -> p c m", p=P))
            mm_ps = ps_wh.tile([P, N], F32, tag="whps")
            for c in range(C):
                nc.tensor.matmul(mm_ps[:, :HID], cw1T_sb[:, c, :],
                                 s_full[:, c, :HID],
                                 start=(c == 0), stop=(c == C - 1))
            hf = sc_32.tile([P, N], F32, tag="s32")
            nc.vector.scalar_tensor_tensor(hf[:, :HID], sf[:], (1.0 - THETA2) / THETA2,
                                           mm_ps[:, :HID], op0=OP.mult, op1=OP.add)
            nc.vector.scalar_tensor_tensor(hf[:, :HID], hf[:, :HID], THETA2, h0f[:],
                                           op0=OP.mult, op1=OP.add)
            nc.vector.scalar_tensor_tensor(hf[:, :HID], hf[:, :HID], SLOPE, hf[:, :HID],
                                           op0=OP.mult, op1=OP.max)
            hb = sc_bf.tile([P, HID], BF16, tag="h0b")
            nc.scalar.activation(hb[:], hf[:, :HID], AF.Copy)
            hT = per.tile([P, 4, P], BF16, tag="hT")
            transpose_into(hT, hb[:], 4)

            fc1_fn = w_stream(d["fc1_w"].ap().rearrange("(c p) f -> p c f", p=P), N)
            y_ps = ps_wh.tile([P, N], F32, tag="whps")
            for c in range(4):
                fc1_c = fc1_fn(c)
                for fh in range(2):
                    nc.tensor.matmul(y_ps[:, fh * 512:(fh + 1) * 512], hT[:, c, :],
                                     fc1_c[:, fh * 512:(fh + 1) * 512],
                                     start=(c == 0), stop=(c == 3))
            b1_bc = abc_p.tile([P, 2 * N], BF16, tag="abc")
            bcast_vec(b1_bc, d["fc1_b"].ap(), N)
            y_sb = sc_32.tile([P, N], F32, tag="s32")
            nc.vector.scalar_tensor_tensor(y_sb[:], y_ps[:], 1.0, b1_bc[:, :N],
                                           op0=OP.mult, op1=OP.add)
            nc.sync.dma_start(out_d.ap(), y_sb[:])


def _shard_inputs(inputs):
    bf = lambda a: np.ascontiguousarray(np.asarray(a, dtype=np.float32)).astype(
        ml_dtypes.bfloat16)
    x = np.asarray(inputs["x"], np.float32)
    adj = np.asarray(inputs["adj"], np.float32)
    x_bf = bf(x)
    xT_bf = np.ascontiguousarray(x_bf.T)
    cw1_bf = bf(inputs["cw1"])
    cw1T = np.ascontiguousarray(cw1_bf.T)
    shared = {
        "Wg1": bf(inputs["Wg1"]),
        "ag1": bf(np.asarray(inputs["ag1"])[:, :, 0]),
        "Wo1": bf(inputs["Wo1"]),
        "ao1": bf(np.asarray(inputs["ao1"])[:, 0]),
        "Wg2": bf(inputs["Wg2"]),
        "ag2": bf(np.asarray(inputs["ag2"])[:, :, 0]),
        "Wo2": bf(inputs["Wo2"]),
        "ao2": bf(np.asarray(inputs["ao2"])[:, 0]),
        "fc0_w": bf(inputs["fc0_w"]),
        "fc0_b": bf(inputs["fc0_b"]),
        "fc1_w": bf(inputs["fc1_w"]),
        "fc1_b": bf(inputs["fc1_b"]),
    }
    in_maps = []
    for c in range(C):
        r0, r1 = c * P, (c + 1) * P
        m = dict(shared)
        m["xT_sl"] = np.ascontiguousarray(xT_bf[:, r0:r1])
        m["adj_r"] = np.ascontiguousarray(adj[r0:r1])
        m["cw1T_sl"] = np.ascontiguousarray(cw1T[:, r0:r1])
        in_maps.append(m)
    return in_maps


def kernel(**inputs) -> np.ndarray:
    if "nc" not in _CACHE:
        _CACHE["nc"] = _build()
    nc = _CACHE["nc"]
    in_maps = _shard_inputs(inputs)
    res = run_bass_kernel_spmd(nc, in_maps, core_ids=list(range(C)))
    out = np.concatenate([res.results[c]["out"] for c in range(C)], axis=0)
    return np.asarray(out, dtype=np.float32)


if __name__ == "__main__":
    rng = np.random.default_rng(0)
    fake = {
        "x": rng.standard_normal((N, N), dtype=np.float32),
        "adj": np.maximum((rng.random((N, N)) < 0.02).astype(np.float32),
                          np.eye(N, dtype=np.float32)),
        "Wg1": rng.standard_normal((H1, N, N), dtype=np.float32) * 0.02,
        "ag1": rng.standard_normal((H1, 2 * N, 1), dtype=np.float32) * 0.02,
        "Wo1": rng.standard_normal((H1 * N, NC1), dtype=np.float32) * 0.02,
        "ao1": rng.standard_normal((2 * NC1, 1), dtype=np.float32) * 0.02,
        "Wg2": rng.standard_normal((H2, NC1, NC1), dtype=np.float32) * 0.02,
        "ag2": rng.standard_normal((H2, 2 * NC1, 1), dtype=np.float32) * 0.02,
        "Wo2": rng.standard_normal((N, N), dtype=np.float32) * 0.02,
        "ao2": rng.standard_normal((2 * N, 1), dtype=np.float32) * 0.02,
        "fc0_w": rng.standard_normal((N, HID), dtype=np.float32) * 0.02,
        "fc0_b": np.zeros(HID, np.float32),
        "fc1_w": rng.standard_normal((HID, N), dtype=np.float32) * 0.02,
        "fc1_b": np.zeros(N, np.float32),
        "cw0": rng.standard_normal((N, N), dtype=np.float32),
        "cw1": rng.standard_normal((N, N), dtype=np.float32),
    }
    y = kernel(**fake)
    print("kernel ran, out shape", y.shape, "finite:", np.isfinite(y).all())
